# revision 1
# baseline (speedup 1.0000x reference)
r"""DbrxAttention on 8 TRN2 NeuronCores, tensor-parallel across heads.

Per-core shard (core c of 8): 6 query heads (q heads 6c..6c+5), kv head c
(replicated per its 6-head query group), plus the matching 768 input
columns of the out-projection. Each core computes a partial out-proj
(row-parallel Wout); the partials are summed on the host (the all-reduce
of the TP pattern).

Layouts (per core, all device tensors):
  hidT   [6144, 2048] fp16  hidden^T       (d on partitions)
  wqkvT  [6144, 1024] fp16  [q0..q5 | k | v] columns of Wqkv^T shard
  woutT  [768,  6144] fp16  Wout[:, shard]^T
  cos/sin tables [128, 2048] fp16, neox rope with sign-folded sin and the
  1/sqrt(128) score scale folded into the q tables.
  masks  [4, 128, 512] fp16  multiplicative causal masks for the four
         diagonal-straddle patterns of (128-wide kt tile, 512-wide qt chunk)

Pipeline: QKV GEMM (fp16, PSUM fp32) -> clip -> rope (DVE + partition-shift
DMA) into fp16 tiles -> scores^T = k^T.T @ q^T per (head, qt-chunk, kt-tile)
block (fp16 MM, software-pipelined 2 deep) -> exp on ACT into fp32r probs ->
causal mask multiply on diagonal blocks -> row sums via ones-matmul + attn^T
accumulation via v-matmul (both fp32r) -> normalization (reciprocal +
partition broadcast) -> fp16 attnT -> out-proj (fp16) -> partial
[2048, 6144] fp32 out, summed across the 8 cores on the host.
"""

import os

import numpy as np

import concourse.mybir as mybir
import concourse.tile as tile
from concourse import bacc
from concourse.bass_utils import run_bass_kernel_spmd

F32R = mybir.dt.float32r
F32 = mybir.dt.float32
F16 = mybir.dt.float16

T = 2048
D = 6144
N_HEADS = 48
N_KV = 8
HD = 128
CLIP = 8.0
THETA = 500000.0
N_CORES = 8
HPC = N_HEADS // N_CORES      # q heads per core = 6
QKJ = HPC + 1                 # q+k j-tiles per core = 7
DCH = D // 128                # 48 contraction chunks
TCH = T // 512                # 4 t-chunks
TTILES = T // 128             # 16 t-tiles
OCH = D // 512                # 12 out-proj column chunks
ICH = HPC                     # 6 out-proj contraction chunks (768/128)

_compiled = None


def _build():
    nc = bacc.Bacc("TRN2", target_bir_lowering=False, debug=False,
                   num_devices=N_CORES)

    hidT_d = nc.dram_tensor("hidT", [D, T], F16, kind="ExternalInput").ap()
    wqkvT_d = nc.dram_tensor("wqkvT", [D, 1024], F16, kind="ExternalInput").ap()
    woutT_d = nc.dram_tensor("woutT", [HPC * HD, D], F16, kind="ExternalInput").ap()
    cosq_d = nc.dram_tensor("cosq", [HD, T], F16, kind="ExternalInput").ap()
    sinq_d = nc.dram_tensor("sinq", [HD, T], F16, kind="ExternalInput").ap()
    cosk_d = nc.dram_tensor("cosk", [HD, T], F16, kind="ExternalInput").ap()
    sink_d = nc.dram_tensor("sink", [HD, T], F16, kind="ExternalInput").ap()
    mask_d = nc.dram_tensor("maskm", [4, HD, 512], F16, kind="ExternalInput").ap()
    ones_d = nc.dram_tensor("ones", [HD, 33], F32R, kind="ExternalInput").ap()
    outp_d = nc.dram_tensor("outp", [T, D], F32, kind="ExternalOutput").ap()

    mn, mx = mybir.AluOpType.min, mybir.AluOpType.max
    mult, add = mybir.AluOpType.mult, mybir.AluOpType.add
    EXP = mybir.ActivationFunctionType.Exp

    with tile.TileContext(nc) as tc:
        with (
            tc.tile_pool(name="sb", bufs=1) as pool,
            tc.tile_pool(name="ps", bufs=1, space="PSUM") as psum,
        ):
            # persistent tensors
            qkT = pool.tile([128, QKJ, T], F16)       # roped q (scaled) + k
            v_sb = pool.tile([128, TTILES, HD], F32R)  # clipped v, [t%128, t//128, hd]
            attnT = pool.tile([128, HPC, T], F16)      # normalized attn^T
            cosq = pool.tile([HD, T], F16)
            sinq = pool.tile([HD, T], F16)
            cosk = pool.tile([HD, T], F16)
            sink = pool.tile([HD, T], F16)
            masks = pool.tile([HD, 4, 512], F16)
            ones = pool.tile([HD, 33], F32R)

            def load_tables():
                nc.gpsimd.dma_start(cosq[:], cosq_d[:])
                nc.gpsimd.dma_start(sinq[:], sinq_d[:])
                nc.gpsimd.dma_start(cosk[:], cosk_d[:])
                nc.gpsimd.dma_start(sink[:], sink_d[:])
                nc.gpsimd.dma_start(masks[:], mask_d.rearrange("a p t -> p a t"))
                nc.gpsimd.dma_start(ones[:], ones_d[:])

            def qkv_sweep(tcx):
                tsl = slice(tcx * 512, (tcx + 1) * 512)
                qk_ps = [psum.tile([128, 512], F32, tag="bank", bufs=8,
                                   name=f"qk_ps{j}")
                         for j in range(QKJ)]
                v_ps = psum.tile([128, 512], F32, tag="bank", bufs=8)
                for d in range(DCH):
                    dsl = slice(d * 128, (d + 1) * 128)
                    hid = pool.tile([128, 512], F16, tag="hid", bufs=12)
                    wq = pool.tile([128, 1024], F16, tag="wq", bufs=12)
                    nc.sync.dma_start(hid[:], hidT_d[dsl, tsl])
                    nc.sync.dma_start(wq[:], wqkvT_d[dsl, :])
                    st, sp = d == 0, d == DCH - 1
                    for j in range(QKJ):
                        nc.tensor.matmul(qk_ps[j][:], wq[:, j * 128:(j + 1) * 128],
                                         hid[:], start=st, stop=sp)
                    for s in range(4):
                        # packed quarter-bank outputs: start=True zeroes the
                        # whole 2KB zero-region, so only the first sub-matmul
                        # of the bank may set it
                        nc.tensor.matmul(v_ps[:, s * 128:(s + 1) * 128],
                                         hid[:, s * 128:(s + 1) * 128],
                                         wq[:, 896:1024],
                                         start=(st and s == 0),
                                         stop=(sp and s == 3),
                                         skip_group_check=True)
                # evacuate: all clips first (each clip releases a PSUM bank
                # for the interleaved attention/next-sweep matmuls), ropes
                # after (they only read the SBUF raw tiles)
                raws = []
                for j in range(QKJ):
                    raw = pool.tile([128, 512], F32, tag="raw", bufs=8,
                                    name=f"raw{j}")
                    nc.vector.tensor_scalar(raw[:], qk_ps[j][:], CLIP, -CLIP, mn, mx)
                    raws.append(raw)
                nc.vector.tensor_scalar(
                    v_sb[:, tcx * 4:(tcx + 1) * 4, :],
                    v_ps[:].rearrange("p (a h) -> p a h", a=4),
                    CLIP, -CLIP, mn, mx)
                for j in [HPC] + list(range(HPC)):
                    raw = raws[j]
                    xr = pool.tile([128, 512], F32, tag="xr", bufs=4)
                    nc.sync.dma_start(xr[0:64, :], raw[64:128, :])
                    nc.sync.dma_start(xr[64:128, :], raw[0:64, :])
                    cosT = cosq if j < HPC else cosk
                    sinT = sinq if j < HPC else sink
                    dst = qkT[:, j, tsl]
                    nc.vector.tensor_tensor(dst, raw[:], cosT[:, tsl], mult)
                    nc.vector.tensor_tensor(xr[:], xr[:], sinT[:, tsl], mult)
                    nc.vector.tensor_tensor(dst, dst, xr[:], add)

            def attn_chain(h, jc):
                qsl = slice(jc * 512, (jc + 1) * 512)
                n_kt = 4 * jc + 4
                attn_ps = psum.tile([128, 512], F32, tag="bank", bufs=8)
                sums_ps = psum.tile([1, 512], F32, tag="bank", bufs=8)
                LEAD = 2
                pbs = {}
                for step in range(n_kt + LEAD):
                    if step < n_kt:
                        kt = step
                        sc = psum.tile([128, 512], F32, tag="bank", bufs=8)
                        nc.tensor.matmul(sc[:],
                                         qkT[:, HPC, kt * 128:(kt + 1) * 128],
                                         qkT[:, h, qsl], start=True, stop=True)
                        pb = pool.tile([128, 512], F32R, tag="pb", bufs=6)
                        nc.scalar.activation(pb[:], sc[:], EXP)
                        r = kt - 4 * jc
                        if r >= 0:
                            nc.vector.tensor_tensor(pb[:], pb[:], masks[:, r, :],
                                                    mult)
                        pbs[kt] = pb
                    if step >= LEAD:
                        kt = step - LEAD
                        pb = pbs.pop(kt)
                        st, sp = kt == 0, kt == n_kt - 1
                        nc.tensor.matmul(sums_ps[:], ones[:, 0:1], pb[:],
                                         start=st, stop=sp)
                        nc.tensor.matmul(attn_ps[:], v_sb[:, kt, :], pb[:],
                                         start=st, stop=sp)
                # release both banks fast (ACT copy + DVE recip), then
                # normalize off the critical path
                au = pool.tile([128, 512], F32, tag="au", bufs=4)
                nc.scalar.copy(au[:], attn_ps[:])
                rec = pool.tile([1, 512], F32, tag="rec", bufs=4)
                nc.vector.reciprocal(rec[:], sums_ps[:])
                recb = pool.tile([128, 512], F32, tag="recb", bufs=4)
                nc.gpsimd.partition_broadcast(recb[:], rec[:])
                nc.vector.tensor_tensor(attnT[:, h, qsl], au[:], recb[:], mult)

            def outproj():
                for oc in range(OCH):
                    osl = slice(oc * 512, (oc + 1) * 512)
                    wo = pool.tile([128, ICH, 512], F16, tag="wo", bufs=3)
                    nc.sync.dma_start(wo[:], woutT_d[:, osl].rearrange(
                        "(i p) o -> p i o", p=128))
                    for t in range(TTILES):
                        out_ps = psum.tile([128, 512], F32, tag="bank", bufs=8)
                        for i in range(ICH):
                            nc.tensor.matmul(out_ps[:],
                                             attnT[:, i, t * 128:(t + 1) * 128],
                                             wo[:, i, :], start=(i == 0),
                                             stop=(i == ICH - 1))
                        osb = pool.tile([128, 512], F32, tag="osb", bufs=4)
                        nc.scalar.copy(osb[:], out_ps[:])
                        nc.sync.dma_start(outp_d[t * 128:(t + 1) * 128, osl], osb[:])

            # ---- Sequential phases; chains jc-outer so the last sweep's
            # rope only gates the final quarter of chains ----
            load_tables()
            for tcx in range(TCH):
                qkv_sweep(tcx)
            for jc in range(TCH):
                for h in range(HPC):
                    attn_chain(h, jc)
            outproj()

    nc.compile()
    return nc


def kernel(hidden_states, position_ids, Wqkv, Wout):
    global _compiled
    hidden_states = np.asarray(hidden_states, dtype=np.float32)
    position_ids = np.asarray(position_ids).astype(np.int64)
    Wqkv = np.asarray(Wqkv, dtype=np.float32)
    Wout = np.asarray(Wout, dtype=np.float32)

    if _compiled is None:
        _compiled = _build()
    nc = _compiled

    # host prep: rope tables (from actual position_ids), masks, shards
    scale = HD ** -0.5
    half = HD // 2
    inv_freq = 1.0 / (THETA ** (np.arange(half, dtype=np.float64) / half))
    freqs = position_ids.astype(np.float64)[None, :] * inv_freq[:, None]  # [64, T]
    cos = np.cos(freqs)
    sin = np.sin(freqs)
    cosf = np.concatenate([cos, cos], 0)
    sinf = np.concatenate([-sin, sin], 0)
    cosq = (cosf * scale).astype(np.float16)
    sinq = (sinf * scale).astype(np.float16)
    cosk = cosf.astype(np.float16)
    sink = sinf.astype(np.float16)

    p = np.arange(128)[:, None]
    f = np.arange(512)[None, :]
    masks = np.stack([(f >= 128 * r + p) for r in range(4)]).astype(np.float16)

    hidT = np.ascontiguousarray(hidden_states.T).astype(np.float16)
    ones = np.ones((HD, 33), np.float32)

    q_size = N_HEADS * HD
    in_maps = []
    for c in range(N_CORES):
        qrows = Wqkv[c * HPC * HD:(c + 1) * HPC * HD]
        krows = Wqkv[q_size + c * HD:q_size + (c + 1) * HD]
        vrows = Wqkv[q_size + N_KV * HD + c * HD:q_size + N_KV * HD + (c + 1) * HD]
        wqkvT = np.ascontiguousarray(
            np.concatenate([qrows, krows, vrows], 0).T).astype(np.float16)
        woutT = np.ascontiguousarray(
            Wout[:, c * HPC * HD:(c + 1) * HPC * HD].T).astype(np.float16)
        in_maps.append({
            "hidT": hidT, "wqkvT": wqkvT, "woutT": woutT,
            "cosq": cosq, "sinq": sinq, "cosk": cosk, "sink": sink,
            "maskm": masks, "ones": ones,
        })

    trace = os.environ.get("DBRX_TRACE", "0") == "1"
    res = run_bass_kernel_spmd(nc, in_maps, core_ids=list(range(N_CORES)),
                               trace=trace)
    kernel.last_result = res

    out = res.results[0]["outp"].astype(np.float32)
    for c in range(1, N_CORES):
        out += res.results[c]["outp"]
    return out

